# revision 11
# baseline (speedup 1.0000x reference)
"""Trainium2 Bass kernel for the attention-LSTM decoder (B=256, T-1=32, ENC=DEC=128, OUT=1).

Sharding: data-parallel, batch 256 -> 32 per core across 8 cores. The T-1=32
recurrence runs locally per core, fully unrolled.

Per-core layouts (Bs=32, tau-major free index j = tau*32 + b):
  - states: H = 2*h [128(dec), 32(b)] bf16, C [128, 32] f32 (+bf16 copy for matmul rhs)
  - P = W1_enc @ enc + b1 precomputed once: [128(h), 1024(j)] bf16
  - F_k = encT_k^T @ Wc^T precomputed once (k=0..7): [128(r), 512(g)] bf16, so the
    gate contribution Wc@ctx comes straight from the attention weights abuf
    (no per-step ctx materialization / PSUM->SBUF copy)
  - per step: q = W1_h@h + W1_c@c (PE) -> pre = P + bcast(q) (DVE) -> hdn = tanh (ACT)
    -> logits via 8 MMs (lhsT = hdn chunk, rhs = W2) into PSUM [128(r), 8(k)]
    -> E = exp (ACT, fused row-sum) -> S = SEL4^T-matmul partition-sum -> 1/S (DVE)
    -> masked attn matrix abuf (DVE)
    -> gates: Whh/Waug MMs issued early (right after q), F@abuf MMs after abuf;
       i,f,o weight rows pre-scaled by 0.5 so ONE tanh over [g|i|f|o] yields
       [tanh(g), t_i, t_f, t_o] (sigmoid(x) = (tanh(x/2)+1)/2)
    -> pointwise: u=(t_i+1)*g~, v=(t_f+1)*c, 2c'=u+v, th=tanh(c'), H=(t_o+1)*th
"""

import numpy as np
import ml_dtypes

import concourse.bass as bass
import concourse.bacc as bacc
import concourse.tile as tile
from concourse import mybir
from concourse.bass_utils import run_bass_kernel_spmd

F32 = mybir.dt.float32
BF16 = mybir.dt.bfloat16
AF = mybir.ActivationFunctionType
OP = mybir.AluOpType

B, T, ENC, DEC = 256, 32, 128, 128
NCORES = 8
BS = B // NCORES  # 32 batch rows per core

import os
KVAR = os.environ.get("KVAR", "full")  # bisect: tanh4 | early | full


def _ap_with(ap_obj, dims):
    """Build an AP with explicit free dims (list of [step, count]) keeping partition dim."""
    return bass.AP(tensor=ap_obj.tensor, offset=ap_obj.offset, ap=[ap_obj.ap[0]] + dims)


def build_program(n_steps=T):
    # Bacc (not plain Bass): its compile() runs move_matmul_waits_to_ldweights +
    # generate_event_semaphores, required because HW instructions hold only ONE
    # semaphore wait each.
    nc = bacc.Bacc()

    # ---- DRAM I/O (per-core shard, host-prepared layouts) ----
    d_encT = nc.dram_tensor("encT", [ENC, T * BS], F32, kind="ExternalInput")
    d_encN = nc.dram_tensor("encN", [128, 8 * ENC], BF16, kind="ExternalInput")
    d_yaug = nc.dram_tensor("yaug", [2, T * BS], BF16, kind="ExternalInput")
    d_ylast = nc.dram_tensor("ylast", [1, BS], F32, kind="ExternalInput")
    d_w1eT = nc.dram_tensor("w1eT", [ENC, 128], F32, kind="ExternalInput")
    d_b1 = nc.dram_tensor("b1", [128, 1], F32, kind="ExternalInput")
    d_w1hT = nc.dram_tensor("w1hT", [DEC, 128], BF16, kind="ExternalInput")
    d_w1cT = nc.dram_tensor("w1cT", [DEC, 128], BF16, kind="ExternalInput")
    d_w2c = nc.dram_tensor("w2c", [128, 1], BF16, kind="ExternalInput")
    d_sel4rep = nc.dram_tensor("sel4rep", [128, 128], F32, kind="ExternalInput")
    d_sel4b = nc.dram_tensor("sel4b", [128, BS], BF16, kind="ExternalInput")
    d_whhT = nc.dram_tensor("whhT", [DEC, 512], BF16, kind="ExternalInput")
    d_wcT = nc.dram_tensor("wcT", [ENC, 512], BF16, kind="ExternalInput")
    d_waug = nc.dram_tensor("waug", [2, 512], BF16, kind="ExternalInput")
    d_fcf = nc.dram_tensor("fcf", [128, 2], BF16, kind="ExternalInput")
    d_fcfb = nc.dram_tensor("fcfb", [1, 1], F32, kind="ExternalInput")
    d_out = nc.dram_tensor("outp", [1, BS], F32, kind="ExternalOutput")

    with tile.TileContext(nc) as tc:
        with (
            tc.tile_pool(name="consts", bufs=1) as consts,
            tc.tile_pool(name="state", bufs=1) as state,
            tc.tile_pool(name="temps", bufs=3) as temps,
            tc.tile_pool(name="psum", bufs=1, space="PSUM") as psum,
        ):
            # ---- load constants ----
            encN = consts.tile([128, 8 * ENC], BF16)
            nc.sync.dma_start(out=encN, in_=d_encN[:, :])
            yaug = consts.tile([2, T * BS], BF16)
            nc.sync.dma_start(out=yaug, in_=d_yaug[:, :])
            ylast = consts.tile([1, BS], F32)
            nc.sync.dma_start(out=ylast, in_=d_ylast[:, :])
            b1 = consts.tile([128, 1], F32)
            nc.sync.dma_start(out=b1, in_=d_b1[:, :])
            w1hT = consts.tile([DEC, 128], BF16)
            nc.sync.dma_start(out=w1hT, in_=d_w1hT[:, :])
            w1cT = consts.tile([DEC, 128], BF16)
            nc.sync.dma_start(out=w1cT, in_=d_w1cT[:, :])
            w2c = consts.tile([128, 1], BF16)
            nc.sync.dma_start(out=w2c, in_=d_w2c[:, :])
            sel4rep = consts.tile([128, 128], F32)
            nc.sync.dma_start(out=sel4rep, in_=d_sel4rep[:, :])
            sel4b = consts.tile([128, BS], BF16)
            nc.sync.dma_start(out=sel4b, in_=d_sel4b[:, :])
            whhT = consts.tile([DEC, 512], BF16)
            nc.sync.dma_start(out=whhT, in_=d_whhT[:, :])
            wcT = consts.tile([ENC, 512], BF16)
            nc.sync.dma_start(out=wcT, in_=d_wcT[:, :])
            waug = consts.tile([2, 512], BF16)
            nc.sync.dma_start(out=waug, in_=d_waug[:, :])
            fcf = consts.tile([128, 2], BF16)
            nc.sync.dma_start(out=fcf, in_=d_fcf[:, :])

            # ---- prologue: P = W1_enc @ enc + b1 -> bf16 [128, 1024];
            #      F_k = encT_k^T @ Wc^T -> bf16 [128, 8*512] ----
            P = consts.tile([128, T * BS], BF16)
            F = consts.tile([128, 8 * 512], BF16)
            with tc.tile_pool(name="prolog", bufs=1) as prolog:
                encT = prolog.tile([ENC, T * BS], F32)
                nc.sync.dma_start(out=encT, in_=d_encT[:, :])
                w1eT = prolog.tile([ENC, 128], F32)
                nc.sync.dma_start(out=w1eT, in_=d_w1eT[:, :])

                # PE sync-fence: walrus Matmult/LDWEIGHTS carries at most ONE
                # semaphore wait. Touch every DMA-loaded tile with a dummy
                # 1x1x1 matmul (both operands in the same tile -> 1 wait each)
                # so no real matmul is first-contact for two sem domains.
                pdum = psum.tile([1, 1], F32, tag="S")
                dscr = prolog.tile([1, 16], F32)
                for i, cst in enumerate((encT, w1eT, encN, yaug, ylast, b1, w1hT,
                                         w1cT, w2c, sel4rep, sel4b, whhT,
                                         wcT, waug, fcf)):
                    nc.tensor.matmul(pdum[:, :], cst[0:1, 0:1], cst[0:1, 0:1],
                                     start=True, stop=True)
                    # same fence for the vector engine (1-wait limit is universal)
                    nc.vector.tensor_copy(dscr[0:1, i:i + 1], cst[0:1, 0:1])

                # bf16 copy of encT for the F matmuls
                encTb = prolog.tile([ENC, T * BS], BF16)
                nc.vector.tensor_copy(encTb, encT[:, :])

                for half in range(2):
                    pP = psum.tile([128, 512], F32, tag="gA" if half == 0 else "gB")
                    nc.tensor.matmul(
                        pP[:, :], w1eT[:, :], encT[:, half * 512:(half + 1) * 512],
                        start=True, stop=True,
                    )
                    # P half = psum + b1 (per-partition scalar), cast to bf16
                    nc.vector.tensor_scalar(
                        out=P[:, half * 512:(half + 1) * 512],
                        in0=pP[:, :], scalar1=b1[:, :], scalar2=None, op0=OP.add,
                    )

                # F_k[r, g] = sum_e enc[b_r, tau_k(r), e] * Wc[g, e]
                # (wcT rows for i,f,o pre-scaled 0.5 host-side)
                for k in range(8):
                    pF = psum.tile([128, 512], F32, tag="gA" if k % 2 == 0 else "gB")
                    nc.tensor.matmul(
                        pF[:, :], encTb[:, k * 128:(k + 1) * 128], wcT[:, :],
                        start=True, stop=True,
                    )
                    # alternate DVE/ACT for the PSUM->SBUF copies to overlap
                    if k % 2 == 0:
                        nc.vector.tensor_copy(F[:, k * 512:(k + 1) * 512], pF[:, :])
                    else:
                        nc.scalar.copy(out=F[:, k * 512:(k + 1) * 512], in_=pF[:, :])

            # ---- state init ----
            H = state.tile([DEC, BS], BF16)   # 2*h
            Cn = state.tile([DEC, BS], F32)   # c
            Cb = state.tile([DEC, BS], BF16)  # bf16 copy of c
            nc.vector.memset(H, 0.0)
            nc.vector.memset(Cn, 0.0)
            nc.vector.memset(Cb, 0.0)

            abuf = None
            for t in range(n_steps):
                # --- phase A: attention MLP; Whh/Waug gate MMs issued early ---
                pq = psum.tile([128, BS], F32, tag="q")
                nc.tensor.matmul(pq[:, :], w1cT[:, :], Cb[:, :], start=True, stop=False)
                nc.tensor.matmul(pq[:, :], w1hT[:, :], H[:, :], start=False, stop=True)

                # halves pipeline: DVE pre-add h1 overlaps ACT tanh h0;
                # logits MMs for chunks 0-3 overlap ACT tanh h1.
                # pre reads q straight from PSUM (1x mode, but saves the
                # PSUM->SBUF copy hop on the critical chain)
                pre = temps.tile([128, T * BS], BF16, tag="pre")
                hdn = temps.tile([128, T * BS], BF16, tag="hdn")
                pL = psum.tile([128, 8], F32, tag="L")
                HW_ = T * BS // 2  # 512 elems = 4 chunks per half
                for h in range(2):
                    q_b = _ap_with(pq[:, :], [[0, T // 2], [1, BS]])
                    nc.vector.tensor_add(
                        pre[:, h * HW_:(h + 1) * HW_].rearrange("p (t b) -> p t b", b=BS),
                        P[:, h * HW_:(h + 1) * HW_].rearrange("p (t b) -> p t b", b=BS),
                        q_b,
                    )
                    nc.scalar.activation(hdn[:, h * HW_:(h + 1) * HW_],
                                         pre[:, h * HW_:(h + 1) * HW_], AF.Tanh)
                    for k in range(4 * h, 4 * h + 4):
                        nc.tensor.matmul(
                            pL[:, k:k + 1], hdn[:, k * 128:(k + 1) * 128], w2c[:, :],
                            start=True, stop=True,
                        )
                # --- phase B: softmax weights ---
                E2 = temps.tile([128, 8], BF16, tag="E2")
                Ered = temps.tile([128, 1], F32, tag="Ered")
                nc.scalar.activation(E2[:, :], pL[:, :], AF.Exp, accum_out=Ered[:, :])
                # replicate+sum in ONE matmul: S128[p] = sum_r [r%32==p%32]*Ered[r]
                pS = psum.tile([128, 1], F32, tag="S")
                nc.tensor.matmul(pS[:, :], sel4rep[:, :], Ered[:, :], start=True, stop=True)
                R128 = temps.tile([128, 1], F32, tag="R128")
                nc.vector.reciprocal(R128[:, :], pS[:, :])

                # unnormalized masked attn matrix: independent of S/recip,
                # overlaps the S-matmul + reciprocal on the other engines
                abuf_u = temps.tile([128, 8 * BS], BF16, tag="abufu")
                e2_b = _ap_with(E2[:, :], [[1, 8], [0, BS]])
                sel_b = _ap_with(sel4b[:, :], [[0, 8], [1, BS]])
                nc.vector.tensor_mul(
                    abuf_u[:, :].rearrange("p (k b) -> p k b", b=BS),
                    e2_b, sel_b,
                )
                # normalize rows by 1/S: cheap per-partition tensor_scalar
                abuf = temps.tile([128, 8 * BS], BF16, tag="abuf")
                nc.vector.tensor_scalar(
                    out=abuf[:, :], in0=abuf_u[:, :], scalar1=R128[:, :],
                    scalar2=None, op0=OP.mult,
                )

                # --- phase C: gate MMs, one CONTIGUOUS accumulation group per
                # slice (PE supports only one open group at a time): Whh@H +
                # Waug@[y;1] + Wc@ctx via F@abuf ---
                pg = psum.tile([128, 4 * BS], F32, tag="g")
                for m in range(4):
                    sl = pg[:, m * BS:(m + 1) * BS]
                    nc.tensor.matmul(sl, whhT[:, m * 128:(m + 1) * 128], H[:, :],
                                     start=True, stop=False)
                    nc.tensor.matmul(sl, waug[:, m * 128:(m + 1) * 128],
                                     yaug[:, t * BS:(t + 1) * BS],
                                     start=False, stop=False)
                    for k in range(8):
                        nc.tensor.matmul(
                            sl,
                            F[:, k * 512 + m * 128:k * 512 + (m + 1) * 128],
                            abuf[:, k * BS:(k + 1) * BS],
                            start=False, stop=(k == 7),
                        )

                # --- LSTM pointwise: ONE tanh over [g|i|f|o] (i,f,o pre-scaled
                # 0.5 in the weights) -> [g~, t_i, t_f, t_o] ---
                T4 = temps.tile([128, 4 * BS], F32, tag="T4")
                nc.scalar.activation(T4[:, :], pg[:, :], AF.Tanh)
                u = temps.tile([128, BS], F32, tag="u")
                nc.vector.scalar_tensor_tensor(
                    out=u[:, :], in0=T4[:, BS:2 * BS], scalar=1.0, in1=T4[:, 0:BS],
                    op0=OP.add, op1=OP.mult)  # (t_i+1)*g~ = 2*sig(i)*g~
                v = temps.tile([128, BS], F32, tag="v")
                nc.vector.scalar_tensor_tensor(
                    out=v[:, :], in0=T4[:, 2 * BS:3 * BS], scalar=1.0, in1=Cn[:, :],
                    op0=OP.add, op1=OP.mult)  # (t_f+1)*c = 2*sig(f)*c
                w2 = temps.tile([128, BS], F32, tag="w2t")
                nc.vector.tensor_add(w2[:, :], u[:, :], v[:, :])  # 2*c_new
                # tanh(c') straight from 2c' (scale=0.5); Cn/Cb updates run
                # off the critical chain in parallel with th/H
                th = temps.tile([128, BS], F32, tag="th")
                nc.scalar.activation(th[:, :], w2[:, :], AF.Tanh, scale=0.5)
                nc.vector.tensor_scalar(out=Cn[:, :], in0=w2[:, :], scalar1=0.5,
                                        scalar2=None, op0=OP.mult)
                nc.vector.tensor_scalar(out=Cb[:, :], in0=w2[:, :], scalar1=0.5,
                                        scalar2=None, op0=OP.mult)
                nc.vector.scalar_tensor_tensor(
                    out=H[:, :], in0=T4[:, 3 * BS:4 * BS], scalar=1.0, in1=th[:, :],
                    op0=OP.add, op1=OP.mult)  # (t_o+1)*tanh(c) = 2*h_new

            # ---- final output: ctx for the last step from the last abuf ----
            pctx = psum.tile([128, BS], F32, tag="ctx")
            for k in range(8):
                nc.tensor.matmul(
                    pctx[:, :], encN[:, k * 128:(k + 1) * 128],
                    abuf[:, k * BS:(k + 1) * BS],
                    start=(k == 0), stop=(k == 7),
                )
            ctx_sb = temps.tile([128, BS], BF16, tag="ctxsb")
            nc.scalar.copy(out=ctx_sb[:, :], in_=pctx[:, :])

            po = psum.tile([1, BS], F32, tag="o")
            nc.tensor.matmul(po[:, :], fcf[:, 0:1], H[:, :], start=True, stop=False)
            nc.tensor.matmul(po[:, :], fcf[:, 1:2], ctx_sb[:, :], start=False, stop=True)
            fcfb = consts.tile([1, 1], F32)
            nc.sync.dma_start(out=fcfb, in_=d_fcfb[:, :])
            out_sb = temps.tile([1, BS], F32, tag="osb")
            nc.vector.scalar_tensor_tensor(
                out=out_sb[:, :], in0=po[:, :], scalar=fcfb[:, :], in1=ylast[:, :],
                op0=OP.add, op1=OP.add)
            nc.sync.dma_start(out=d_out[:, :], in_=out_sb[:, :])

    nc.compile()
    return nc


def _prep_inputs(input_encoded, y_history, attn_W1, attn_b1, attn_W2, attn_b2,
                 W_ih, W_hh, b_ih, b_hh, fc_W, fc_b, fcf_W, fcf_b):
    """Host-side weight fusion + per-core shard layout prep (numpy only)."""
    f32 = np.float32
    bf16 = ml_dtypes.bfloat16
    input_encoded = np.asarray(input_encoded, f32)
    y_history = np.asarray(y_history, f32)

    # attention weights
    W1 = np.asarray(attn_W1, f32)            # [128, 384] cols: h, c, enc
    w1hT = np.ascontiguousarray((0.5 * W1[:, 0:128]).T)     # H = 2h
    w1cT = np.ascontiguousarray(W1[:, 128:256].T)
    w1eT = np.ascontiguousarray(W1[:, 256:384].T)
    b1 = np.asarray(attn_b1, f32).reshape(128, 1)
    w2c = np.asarray(attn_W2, f32).reshape(1, 128).T.copy()  # [128,1]

    # fused gate weights; reorder (i,f,g,o) -> (g,i,f,o); the i,f,o rows get
    # an extra 0.5 so one tanh produces [tanh(g), tanh(i/2), tanh(f/2), tanh(o/2)]
    W_ih = np.asarray(W_ih, f32)
    W_hh = np.asarray(W_hh, f32)
    fc_W = np.asarray(fc_W, f32)
    wc_full = np.outer(W_ih[:, 0], fc_W[0, :128])            # [512, 128]
    w_y = W_ih[:, 0] * fc_W[0, 128]
    bias_g = np.asarray(b_ih, f32) + np.asarray(b_hh, f32) + W_ih[:, 0] * f32(fc_b[0])
    perm = np.r_[256:384, 0:128, 128:256, 384:512]
    gsc = np.ones((512,), f32)
    gsc[128:] = 0.5                                          # i,f,o blocks post-perm
    whhT = np.ascontiguousarray((0.5 * W_hh[perm] * gsc[:, None]).T)  # [128, 512]
    wcT = np.ascontiguousarray((wc_full[perm] * gsc[:, None]).T)      # [128, 512]
    waug = np.stack([w_y[perm] * gsc, bias_g[perm] * gsc], 0)         # [2, 512]

    fcf_W = np.asarray(fcf_W, f32)
    fcf = np.stack([0.5 * fcf_W[0, 0:128], fcf_W[0, 128:256]], 1)  # [128, 2]
    fcfb = np.array([[np.asarray(fcf_b, f32).reshape(-1)[0]]], f32)

    # selection matrices: sel4[r, b] = (r % 32 == b); sel4rep[r, p] = (r%32 == p%32)
    r = np.arange(128)
    sel4 = (np.equal.outer(r % BS, np.arange(BS))).astype(f32)  # [128, 32]
    sel4rep = (np.equal.outer(r % BS, np.arange(128) % BS)).astype(f32)  # [128, 128]

    shared = dict(
        w1eT=w1eT, b1=b1,
        w1hT=w1hT.astype(bf16), w1cT=w1cT.astype(bf16), w2c=w2c.astype(bf16),
        sel4rep=sel4rep, sel4b=sel4.astype(bf16),
        whhT=whhT.astype(bf16), wcT=wcT.astype(bf16), waug=waug.astype(bf16),
        fcf=fcf.astype(bf16), fcfb=fcfb,
    )

    in_maps = []
    for c in range(NCORES):
        enc_c = input_encoded[c * BS:(c + 1) * BS]           # [32, 32, 128]
        y_c = y_history[c * BS:(c + 1) * BS, :, 0]           # [32b, 32tau]
        encT = np.ascontiguousarray(enc_c.transpose(2, 1, 0).reshape(ENC, T * BS))
        # encN[r, k*128+e] = enc[b=r%32, tau=4k+r//32, e]
        tmp = enc_c.transpose(1, 0, 2).reshape(8, 4, BS, ENC)   # [k, tau_lo, b, e]
        encN = np.ascontiguousarray(tmp.transpose(1, 2, 0, 3).reshape(128, 8 * ENC))
        yrow = np.ascontiguousarray(y_c.T.reshape(1, T * BS))   # [1, tau*32+b]
        yaug = np.concatenate([yrow, np.ones_like(yrow)], 0)    # [2, 1024]
        m = dict(shared)
        m.update(
            encT=encT, encN=encN.astype(bf16), yaug=yaug.astype(bf16),
            ylast=np.ascontiguousarray(y_c[:, T - 1].reshape(1, BS)),
        )
        in_maps.append(m)
    return in_maps


_CACHED = {}


def kernel(**inputs) -> np.ndarray:
    in_maps = _prep_inputs(**inputs)
    if "nc" not in _CACHED:
        _CACHED["nc"] = build_program()
    res = run_bass_kernel_spmd(_CACHED["nc"], in_maps, core_ids=list(range(NCORES)))
    out = np.concatenate([r["outp"].reshape(BS, 1) for r in res.results], 0)
    return out.astype(np.float32)


if __name__ == "__main__":
    import reference
    inputs = {k: np.asarray(v) for k, v in reference.setup_inputs().items()}
    expected = np.asarray(reference.reference(**inputs))
    actual = kernel(**inputs)
    err = np.abs(actual - expected).max() / (np.abs(expected).max() + 1e-12)
    print("Relative error:", err)


# revision 15
# speedup vs baseline: 1.1235x; 1.1235x over previous
"""Trainium2 Bass kernel for the attention-LSTM decoder (B=256, T-1=32, ENC=DEC=128, OUT=1).

Sharding: data-parallel, batch 256 -> 32 per core across 8 cores. On each core
the 32-batch is split into TWO independent 16-batch chains, software-pipelined
with a half-step phase offset: chain A's ACT-heavy tail (tanh4/th) overlaps
chain B's DVE/PE-heavy head (q/pre/tanh) and vice versa. The T-1=32 recurrence
is fully unrolled.

Per-chain layouts (Bs=16, r = tau_lo*16 + b with tau = 8k + tau_lo, k=0..3):
  - states: H = 2*h [128(dec), 16(b)] bf16, C [128, 16] f32 (+bf16 copy)
  - P_c = W1_enc @ enc + b1 precomputed: [128(h), 512(j=tau*16+b)] bf16
  - F_ck = encT_ck^T @ Wc^T precomputed: [128(r), 512(g)] bf16 so the gate
    contribution Wc@ctx comes straight from the attention weights abuf
  - per step: q = W1_h@h + W1_c@c (PE) -> pre = P + bcast(q) (DVE) -> tanh (ACT)
    -> logits via 4 MMs (lhsT = hdn chunk, rhs = W2) -> exp (ACT, fused row-sum)
    -> S via SEL16-matmul partition sum -> 1/S -> masked attn matrix abuf (DVE)
    -> gates = Whh@H + Waug@[y;1] + sum_k F_k@abuf_k (one contiguous PSUM
       accumulation group per gate block; PE allows only ONE open group)
    -> ONE tanh over [g|i|f|o] (i,f,o weight rows pre-scaled 0.5) ->
       u=(t_i+1)*g~, v=(t_f+1)*c, 2c'=u+v, th=tanh(c'), H=(t_o+1)*th
"""

import numpy as np
import ml_dtypes

import concourse.bass as bass
import concourse.bacc as bacc
import concourse.tile as tile
from concourse import mybir
from concourse.bass_utils import run_bass_kernel_spmd

F32 = mybir.dt.float32
BF16 = mybir.dt.bfloat16
AF = mybir.ActivationFunctionType
OP = mybir.AluOpType

B, T, ENC, DEC = 256, 32, 128, 128
NCORES = 8
BS = B // NCORES   # 32 batch rows per core
BC = BS // 2       # 16 batch rows per chain
NK = T * BC // 128  # 4 tau-chunks per chain


def _ap_with(ap_obj, dims):
    """Build an AP with explicit free dims (list of [step, count]) keeping partition dim."""
    return bass.AP(tensor=ap_obj.tensor, offset=ap_obj.offset, ap=[ap_obj.ap[0]] + dims)


def build_program(n_steps=T):
    # Bacc (not plain Bass): its compile() runs move_matmul_waits_to_ldweights +
    # generate_event_semaphores, required because HW instructions hold only ONE
    # semaphore wait each.
    nc = bacc.Bacc()

    # ---- DRAM I/O (per-core shard, host-prepared layouts) ----
    d_encT = nc.dram_tensor("encT", [ENC, T * BS], F32, kind="ExternalInput")
    d_encN = nc.dram_tensor("encN", [128, 2 * NK * ENC], BF16, kind="ExternalInput")
    d_yaug = nc.dram_tensor("yaug", [2, T * BS], BF16, kind="ExternalInput")
    d_ylast = nc.dram_tensor("ylast", [1, BS], F32, kind="ExternalInput")
    d_w1eT = nc.dram_tensor("w1eT", [ENC, 128], F32, kind="ExternalInput")
    d_b1 = nc.dram_tensor("b1", [128, 1], F32, kind="ExternalInput")
    d_w1hT = nc.dram_tensor("w1hT", [DEC, 128], BF16, kind="ExternalInput")
    d_w1cT = nc.dram_tensor("w1cT", [DEC, 128], BF16, kind="ExternalInput")
    d_w2c = nc.dram_tensor("w2c", [128, 1], BF16, kind="ExternalInput")
    d_selrep = nc.dram_tensor("selrep", [128, 128], F32, kind="ExternalInput")
    d_selb = nc.dram_tensor("selb", [128, BC], BF16, kind="ExternalInput")
    d_whhT = nc.dram_tensor("whhT", [DEC, 512], BF16, kind="ExternalInput")
    d_wcT = nc.dram_tensor("wcT", [ENC, 512], BF16, kind="ExternalInput")
    d_waug = nc.dram_tensor("waug", [2, 512], BF16, kind="ExternalInput")
    d_fcf = nc.dram_tensor("fcf", [128, 2], BF16, kind="ExternalInput")
    d_fcfb = nc.dram_tensor("fcfb", [1, 1], F32, kind="ExternalInput")
    d_out = nc.dram_tensor("outp", [1, BS], F32, kind="ExternalOutput")

    with tile.TileContext(nc) as tc:
        with (
            tc.tile_pool(name="consts", bufs=1) as consts,
            tc.tile_pool(name="state", bufs=1) as state,
            tc.tile_pool(name="temps", bufs=3) as temps,
            tc.tile_pool(name="psum", bufs=1, space="PSUM") as psum,
        ):
            # all PSUM tags are allocated as full [128, 512] banks and sliced,
            # so a tag can serve differently-shaped uses (prologue reuses gA/gB)
            def pbank(tag):
                return psum.tile([128, 512], F32, tag=tag, name="pb_" + tag)

            # ---- load constants ----
            encN = consts.tile([128, 2 * NK * ENC], BF16)
            nc.sync.dma_start(out=encN, in_=d_encN[:, :])
            yaug = consts.tile([2, T * BS], BF16)
            nc.sync.dma_start(out=yaug, in_=d_yaug[:, :])
            ylast = consts.tile([1, BS], F32)
            nc.sync.dma_start(out=ylast, in_=d_ylast[:, :])
            b1 = consts.tile([128, 1], F32)
            nc.sync.dma_start(out=b1, in_=d_b1[:, :])
            w1hT = consts.tile([DEC, 128], BF16)
            nc.sync.dma_start(out=w1hT, in_=d_w1hT[:, :])
            w1cT = consts.tile([DEC, 128], BF16)
            nc.sync.dma_start(out=w1cT, in_=d_w1cT[:, :])
            w2c = consts.tile([128, 1], BF16)
            nc.sync.dma_start(out=w2c, in_=d_w2c[:, :])
            selrep = consts.tile([128, 128], F32)
            nc.sync.dma_start(out=selrep, in_=d_selrep[:, :])
            selb = consts.tile([128, BC], BF16)
            nc.sync.dma_start(out=selb, in_=d_selb[:, :])
            whhT = consts.tile([DEC, 512], BF16)
            nc.sync.dma_start(out=whhT, in_=d_whhT[:, :])
            wcT = consts.tile([ENC, 512], BF16)
            nc.sync.dma_start(out=wcT, in_=d_wcT[:, :])
            waug = consts.tile([2, 512], BF16)
            nc.sync.dma_start(out=waug, in_=d_waug[:, :])
            fcf = consts.tile([128, 2], BF16)
            nc.sync.dma_start(out=fcf, in_=d_fcf[:, :])
            fcfb = consts.tile([1, 1], F32)
            nc.sync.dma_start(out=fcfb, in_=d_fcfb[:, :])

            # ---- prologue: P = W1_enc @ enc + b1 (chain-major halves);
            #      F_ck = encT_ck^T @ Wc^T ----
            P = consts.tile([128, T * BS], BF16)       # [:, 512c + tau*16 + b]
            F = consts.tile([128, 2 * NK * 512], BF16)  # [:, 2048c + 512k + g]
            with tc.tile_pool(name="prolog", bufs=1) as prolog:
                encT = prolog.tile([ENC, T * BS], F32)
                nc.sync.dma_start(out=encT, in_=d_encT[:, :])
                w1eT = prolog.tile([ENC, 128], F32)
                nc.sync.dma_start(out=w1eT, in_=d_w1eT[:, :])

                # PE sync-fence: walrus Matmult/LDWEIGHTS carries at most ONE
                # semaphore wait. Touch every DMA-loaded tile with a dummy
                # 1x1x1 matmul (both operands in the same tile -> 1 wait each)
                # so no real matmul is first-contact for two sem domains.
                pdum = pbank("o")
                dscr = prolog.tile([1, 16], F32)
                for i, cst in enumerate((encT, w1eT, encN, yaug, ylast, b1, w1hT,
                                         w1cT, w2c, selrep, selb, whhT,
                                         wcT, waug, fcf, fcfb)):
                    nc.tensor.matmul(pdum[0:1, 0:1], cst[0:1, 0:1], cst[0:1, 0:1],
                                     start=True, stop=True)
                    # same fence for the vector engine (1-wait limit is universal)
                    nc.vector.tensor_copy(dscr[0:1, i:i + 1], cst[0:1, 0:1])

                # bf16 copy of encT for the F matmuls
                encTb = prolog.tile([ENC, T * BS], BF16)
                nc.vector.tensor_copy(encTb, encT[:, :])

                for half in range(2):
                    pP = pbank("gA" if half == 0 else "gB")
                    nc.tensor.matmul(
                        pP[:, :], w1eT[:, :], encT[:, half * 512:(half + 1) * 512],
                        start=True, stop=True,
                    )
                    # P half = psum + b1 (per-partition scalar), cast to bf16
                    nc.vector.tensor_scalar(
                        out=P[:, half * 512:(half + 1) * 512],
                        in0=pP[:, :], scalar1=b1[:, :], scalar2=None, op0=OP.add,
                    )

                # F_ck[r, g] = sum_e enc[b_r, tau_k(r), e] * Wc[g, e]
                # (wcT rows for i,f,o pre-scaled 0.5 host-side)
                for kk in range(2 * NK):
                    pF = pbank("gA" if kk % 2 == 0 else "gB")
                    nc.tensor.matmul(
                        pF[:, :], encTb[:, kk * 128:(kk + 1) * 128], wcT[:, :],
                        start=True, stop=True,
                    )
                    # alternate DVE/ACT for the PSUM->SBUF copies to overlap
                    if kk % 2 == 0:
                        nc.vector.tensor_copy(F[:, kk * 512:(kk + 1) * 512], pF[:, :])
                    else:
                        nc.scalar.copy(out=F[:, kk * 512:(kk + 1) * 512], in_=pF[:, :])

            # ---- per-chain state + stage emitters ----
            chains = []
            for c in range(2):
                s = dict(c=c)
                s["H"] = state.tile([DEC, BC], BF16, name="H%d" % c)
                s["Cn"] = state.tile([DEC, BC], F32, name="Cn%d" % c)
                s["Cb"] = state.tile([DEC, BC], BF16, name="Cb%d" % c)
                nc.vector.memset(s["H"], 0.0)
                nc.vector.memset(s["Cn"], 0.0)
                nc.vector.memset(s["Cb"], 0.0)
                s["P"] = P[:, c * 512:(c + 1) * 512]
                s["F0"] = c * 2048          # F col base
                s["encN0"] = c * NK * ENC   # encN col base
                s["yaug0"] = c * 512        # yaug col base
                chains.append(s)

            A, Bc = chains
            QT, GT, LT = ("qA", "qB"), ("gA", "gB"), ("LA", "LB")

            def st_q(s):
                # q = W1c@Cb + W1h@H -> PSUM
                pq = pbank(QT[s["c"]])
                nc.tensor.matmul(pq[:, 0:BC], w1cT[:, :], s["Cb"][:, :],
                                 start=True, stop=False)
                nc.tensor.matmul(pq[:, 0:BC], w1hT[:, :], s["H"][:, :],
                                 start=False, stop=True)
                s["pq"] = pq

            def st_qsb(s):
                q_sb = temps.tile([128, BC], BF16, tag="qsb%d" % s["c"])
                nc.vector.tensor_copy(q_sb[:, :], s["pq"][:, 0:BC])
                s["q_sb"] = q_sb
                s["pre"] = temps.tile([128, T * BC], BF16, tag="pre%d" % s["c"], name="pre%d" % s["c"])
                s["hdn"] = temps.tile([128, T * BC], BF16, tag="hdn%d" % s["c"], name="hdn%d" % s["c"])
                s["pL"] = pbank(LT[s["c"]])

            def st_pre(s, h):
                HW_ = T * BC // 2  # 256
                q_b = _ap_with(s["q_sb"][:, :], [[0, T // 2], [1, BC]])
                nc.vector.tensor_add(
                    s["pre"][:, h * HW_:(h + 1) * HW_].rearrange(
                        "p (t b) -> p t b", b=BC),
                    s["P"][:, h * HW_:(h + 1) * HW_].rearrange(
                        "p (t b) -> p t b", b=BC),
                    q_b,
                )

            def st_tanhlog(s, h):
                HW_ = T * BC // 2
                nc.scalar.activation(s["hdn"][:, h * HW_:(h + 1) * HW_],
                                     s["pre"][:, h * HW_:(h + 1) * HW_], AF.Tanh)
                for k in range(2 * h, 2 * h + 2):
                    nc.tensor.matmul(
                        s["pL"][:, k:k + 1],
                        s["hdn"][:, k * 128:(k + 1) * 128], w2c[:, :],
                        start=True, stop=True,
                    )

            def st_exp(s):
                E2 = temps.tile([128, NK], BF16, tag="E2%d" % s["c"])
                Ered = temps.tile([128, 1], F32, tag="Ered%d" % s["c"])
                nc.scalar.activation(E2[:, :], s["pL"][:, 0:NK], AF.Exp,
                                     accum_out=Ered[:, :])
                s["E2"], s["Ered"] = E2, Ered

            def st_smm(s):
                # S128[p] = sum_r [r%16==p%16]*Ered[r], into col NK of the L bank
                nc.tensor.matmul(s["pL"][:, NK:NK + 1], selrep[:, :],
                                 s["Ered"][:, :], start=True, stop=True)

            def st_norm(s):
                abuf_u = temps.tile([128, NK * BC], BF16, tag="abufu%d" % s["c"])
                e2_b = _ap_with(s["E2"][:, :], [[1, NK], [0, BC]])
                sel_b = _ap_with(selb[:, :], [[0, NK], [1, BC]])
                nc.vector.tensor_mul(
                    abuf_u[:, :].rearrange("p (k b) -> p k b", b=BC),
                    e2_b, sel_b,
                )
                R = temps.tile([128, 1], F32, tag="R%d" % s["c"])
                nc.vector.reciprocal(R[:, :], s["pL"][:, NK:NK + 1])
                abuf = temps.tile([128, NK * BC], BF16, tag="abuf%d" % s["c"])
                nc.vector.tensor_scalar(
                    out=abuf[:, :], in0=abuf_u[:, :], scalar1=R[:, :],
                    scalar2=None, op0=OP.mult,
                )
                s["abuf"] = abuf

            def st_gates(s, t):
                # one CONTIGUOUS accumulation group per gate block (PE allows
                # only one open group at a time -> no foreign MMs in between)
                pg = pbank(GT[s["c"]])
                y0 = s["yaug0"] + t * BC
                for m in range(4):
                    sl = pg[:, m * BC:(m + 1) * BC]
                    nc.tensor.matmul(sl, whhT[:, m * 128:(m + 1) * 128],
                                     s["H"][:, :], start=True, stop=False)
                    nc.tensor.matmul(sl, waug[:, m * 128:(m + 1) * 128],
                                     yaug[:, y0:y0 + BC], start=False, stop=False)
                    for k in range(NK):
                        nc.tensor.matmul(
                            sl,
                            F[:, s["F0"] + k * 512 + m * 128:
                                 s["F0"] + k * 512 + (m + 1) * 128],
                            s["abuf"][:, k * BC:(k + 1) * BC],
                            start=False, stop=(k == NK - 1),
                        )
                s["pg"] = pg

            def st_tanh4(s):
                T4 = temps.tile([128, 4 * BC], F32, tag="T4%d" % s["c"])
                nc.scalar.activation(T4[:, :], s["pg"][:, 0:4 * BC], AF.Tanh)
                s["T4"] = T4

            def st_uvw(s):
                T4 = s["T4"]
                u = temps.tile([128, BC], F32, tag="u%d" % s["c"])
                nc.vector.scalar_tensor_tensor(
                    out=u[:, :], in0=T4[:, BC:2 * BC], scalar=1.0,
                    in1=T4[:, 0:BC], op0=OP.add, op1=OP.mult)
                v = temps.tile([128, BC], F32, tag="v%d" % s["c"])
                nc.vector.scalar_tensor_tensor(
                    out=v[:, :], in0=T4[:, 2 * BC:3 * BC], scalar=1.0,
                    in1=s["Cn"][:, :], op0=OP.add, op1=OP.mult)
                w2 = temps.tile([128, BC], F32, tag="w2%d" % s["c"])
                nc.vector.tensor_add(w2[:, :], u[:, :], v[:, :])  # 2*c_new
                s["w2"] = w2

            def st_th(s):
                th = temps.tile([128, BC], F32, tag="th%d" % s["c"])
                nc.scalar.activation(th[:, :], s["w2"][:, :], AF.Tanh, scale=0.5)
                s["th"] = th

            def st_cncb(s):
                nc.vector.tensor_scalar(out=s["Cn"][:, :], in0=s["w2"][:, :],
                                        scalar1=0.5, scalar2=None, op0=OP.mult)
                nc.vector.tensor_scalar(out=s["Cb"][:, :], in0=s["w2"][:, :],
                                        scalar1=0.5, scalar2=None, op0=OP.mult)

            def st_H(s):
                nc.vector.scalar_tensor_tensor(
                    out=s["H"][:, :], in0=s["T4"][:, 3 * BC:4 * BC], scalar=1.0,
                    in1=s["th"][:, :], op0=OP.add, op1=OP.mult)

            # ---- software-pipelined main loop: B runs half a step behind A ----
            for t in range(n_steps):
                st_q(A)
                if t:
                    st_gates(Bc, t - 1)
                st_qsb(A)
                if t:
                    st_tanh4(Bc)
                st_pre(A, 0)
                if t:
                    st_uvw(Bc)
                st_tanhlog(A, 0)
                if t:
                    st_th(Bc)
                st_pre(A, 1)
                if t:
                    st_cncb(Bc)
                st_tanhlog(A, 1)
                if t:
                    st_H(Bc)
                st_exp(A)
                st_q(Bc)
                st_smm(A)
                st_norm(A)
                st_qsb(Bc)
                st_gates(A, t)
                st_pre(Bc, 0)
                st_tanh4(A)
                st_tanhlog(Bc, 0)
                st_uvw(A)
                st_pre(Bc, 1)
                st_th(A)
                st_tanhlog(Bc, 1)
                st_cncb(A)
                st_H(A)
                st_exp(Bc)
                st_smm(Bc)
                st_norm(Bc)
            # drain chain B's tail for the last step
            st_gates(Bc, n_steps - 1)
            st_tanh4(Bc)
            st_uvw(Bc)
            st_th(Bc)
            st_cncb(Bc)
            st_H(Bc)

            # ---- final output: ctx for the last step from each chain's abuf ----
            out_sb = temps.tile([1, BS], F32, tag="osb")
            pctx = pbank("ctx")
            po = pbank("o")
            for s in chains:
                c = s["c"]
                csl = pctx[:, c * BC:(c + 1) * BC]
                for k in range(NK):
                    nc.tensor.matmul(
                        csl, encN[:, s["encN0"] + k * 128:s["encN0"] + (k + 1) * 128],
                        s["abuf"][:, k * BC:(k + 1) * BC],
                        start=(k == 0), stop=(k == NK - 1),
                    )
                ctx_sb = temps.tile([128, BC], BF16, tag="ctxsb%d" % c)
                nc.scalar.copy(out=ctx_sb[:, :], in_=csl)
                osl = po[0:1, c * BC:(c + 1) * BC]
                nc.tensor.matmul(osl, fcf[:, 0:1], s["H"][:, :],
                                 start=True, stop=False)
                nc.tensor.matmul(osl, fcf[:, 1:2], ctx_sb[:, :],
                                 start=False, stop=True)
                nc.vector.scalar_tensor_tensor(
                    out=out_sb[0:1, c * BC:(c + 1) * BC], in0=osl,
                    scalar=fcfb[:, :], in1=ylast[0:1, c * BC:(c + 1) * BC],
                    op0=OP.add, op1=OP.add)
            nc.sync.dma_start(out=d_out[:, :], in_=out_sb[:, :])

    nc.compile()
    return nc


def _prep_inputs(input_encoded, y_history, attn_W1, attn_b1, attn_W2, attn_b2,
                 W_ih, W_hh, b_ih, b_hh, fc_W, fc_b, fcf_W, fcf_b):
    """Host-side weight fusion + per-core shard layout prep (numpy only)."""
    f32 = np.float32
    bf16 = ml_dtypes.bfloat16
    input_encoded = np.asarray(input_encoded, f32)
    y_history = np.asarray(y_history, f32)

    # attention weights
    W1 = np.asarray(attn_W1, f32)            # [128, 384] cols: h, c, enc
    w1hT = np.ascontiguousarray((0.5 * W1[:, 0:128]).T)     # H = 2h
    w1cT = np.ascontiguousarray(W1[:, 128:256].T)
    w1eT = np.ascontiguousarray(W1[:, 256:384].T)
    b1 = np.asarray(attn_b1, f32).reshape(128, 1)
    w2c = np.asarray(attn_W2, f32).reshape(1, 128).T.copy()  # [128,1]

    # fused gate weights; reorder (i,f,g,o) -> (g,i,f,o); the i,f,o rows get
    # an extra 0.5 so one tanh produces [tanh(g), tanh(i/2), tanh(f/2), tanh(o/2)]
    W_ih = np.asarray(W_ih, f32)
    W_hh = np.asarray(W_hh, f32)
    fc_W = np.asarray(fc_W, f32)
    wc_full = np.outer(W_ih[:, 0], fc_W[0, :128])            # [512, 128]
    w_y = W_ih[:, 0] * fc_W[0, 128]
    bias_g = np.asarray(b_ih, f32) + np.asarray(b_hh, f32) + W_ih[:, 0] * f32(fc_b[0])
    perm = np.r_[256:384, 0:128, 128:256, 384:512]
    gsc = np.ones((512,), f32)
    gsc[128:] = 0.5                                          # i,f,o blocks post-perm
    whhT = np.ascontiguousarray((0.5 * W_hh[perm] * gsc[:, None]).T)  # [128, 512]
    wcT = np.ascontiguousarray((wc_full[perm] * gsc[:, None]).T)      # [128, 512]
    waug = np.stack([w_y[perm] * gsc, bias_g[perm] * gsc], 0)         # [2, 512]

    fcf_W = np.asarray(fcf_W, f32)
    fcf = np.stack([0.5 * fcf_W[0, 0:128], fcf_W[0, 128:256]], 1)  # [128, 2]
    fcfb = np.array([[np.asarray(fcf_b, f32).reshape(-1)[0]]], f32)

    # selection matrices (per-chain, Bs=16): selb[r, b] = (r % 16 == b);
    # selrep[r, p] = (r%16 == p%16)
    r = np.arange(128)
    selb = (np.equal.outer(r % BC, np.arange(BC))).astype(f32)          # [128, 16]
    selrep = (np.equal.outer(r % BC, np.arange(128) % BC)).astype(f32)  # [128, 128]

    shared = dict(
        w1eT=w1eT, b1=b1,
        w1hT=w1hT.astype(bf16), w1cT=w1cT.astype(bf16), w2c=w2c.astype(bf16),
        selrep=selrep, selb=selb.astype(bf16),
        whhT=whhT.astype(bf16), wcT=wcT.astype(bf16), waug=waug.astype(bf16),
        fcf=fcf.astype(bf16), fcfb=fcfb,
    )

    in_maps = []
    for c in range(NCORES):
        enc_c = input_encoded[c * BS:(c + 1) * BS]           # [32b, 32tau, 128e]
        y_c = y_history[c * BS:(c + 1) * BS, :, 0]           # [32b, 32tau]
        # chain-major column order: col = ch*512 + tau*16 + b16
        ec2 = enc_c.reshape(2, BC, T, ENC)                   # [ch, b16, tau, e]
        encT = np.ascontiguousarray(
            ec2.transpose(3, 0, 2, 1).reshape(ENC, T * BS))  # [e, ch*tau*b]
        # encN[r, ch*512 + k*128 + e] = enc[b16=r%16 (+16ch), tau=8k+r//16, e]
        tmp = ec2.reshape(2, BC, NK, 8, ENC)                 # [ch, b, k, tau_lo, e]
        encN = np.ascontiguousarray(
            tmp.transpose(0, 2, 3, 1, 4)                     # [ch, k, tau_lo, b, e]
            .reshape(2, NK, 128, ENC)
            .transpose(0, 2, 1, 3)                           # [ch, r, k, e]
            .reshape(2, 128, NK * ENC)
            .transpose(1, 0, 2)                              # [r, ch, k*e]
            .reshape(128, 2 * NK * ENC))
        y2 = y_c.reshape(2, BC, T)                           # [ch, b16, tau]
        yrow = np.ascontiguousarray(
            y2.transpose(0, 2, 1).reshape(1, T * BS))        # [1, ch*tau*b]
        yaug = np.concatenate([yrow, np.ones_like(yrow)], 0)  # [2, 1024]
        m = dict(shared)
        m.update(
            encT=encT, encN=encN.astype(bf16), yaug=yaug.astype(bf16),
            ylast=np.ascontiguousarray(y_c[:, T - 1].reshape(1, BS)),
        )
        in_maps.append(m)
    return in_maps


_CACHED = {}


def kernel(**inputs) -> np.ndarray:
    in_maps = _prep_inputs(**inputs)
    if "nc" not in _CACHED:
        _CACHED["nc"] = build_program()
    res = run_bass_kernel_spmd(_CACHED["nc"], in_maps, core_ids=list(range(NCORES)))
    out = np.concatenate([r["outp"].reshape(BS, 1) for r in res.results], 0)
    return out.astype(np.float32)


if __name__ == "__main__":
    import reference
    inputs = {k: np.asarray(v) for k, v in reference.setup_inputs().items()}
    expected = np.asarray(reference.reference(**inputs))
    actual = kernel(**inputs)
    err = np.abs(actual - expected).max() / (np.abs(expected).max() + 1e-12)
    print("Relative error:", err)


# revision 17
# speedup vs baseline: 1.1577x; 1.0305x over previous
"""Trainium2 Bass kernel for the attention-LSTM decoder (B=256, T-1=32, ENC=DEC=128, OUT=1).

Sharding: data-parallel, batch 256 -> 32 per core across 8 cores. On each core
the 32-batch is split into TWO independent 16-batch chains, software-pipelined
with a half-step phase offset: chain A's ACT-heavy tail (tanh4/th) overlaps
chain B's DVE/PE-heavy head (q/pre/tanh) and vice versa. The T-1=32 recurrence
is fully unrolled.

Per-chain layouts (Bs=16, r = tau_lo*16 + b with tau = 8k + tau_lo, k=0..3):
  - states: H = 2*h [128(dec), 16(b)] bf16, C [128, 16] f32 (+bf16 copy)
  - P_c = W1_enc @ enc + b1 precomputed: [128(h), 512(j=tau*16+b)] bf16
  - F_ck = encT_ck^T @ Wc^T precomputed: [128(r), 512(g)] bf16 so the gate
    contribution Wc@ctx comes straight from the attention weights abuf
  - per step: q = W1_h@h + W1_c@c (PE) -> pre = P + bcast(q) (DVE) -> tanh (ACT)
    -> logits via 4 MMs (lhsT = hdn chunk, rhs = W2) -> exp (ACT, fused row-sum)
    -> S via SEL16-matmul partition sum -> 1/S -> masked attn matrix abuf (DVE)
    -> gates = Whh@H + Waug@[y;1] + sum_k F_k@abuf_k (one contiguous PSUM
       accumulation group per gate block; PE allows only ONE open group)
    -> ONE tanh over [o|i|f|g] (o,i,f weight rows pre-scaled 0.5) into a state
       tile whose tail holds Cn, so ONE stt yields [u|v]=(T4[i,f]+1)*[g~,c];
       2c'=u+v, th=tanh(c'), H=(t_o+1)*th
"""

import numpy as np
import ml_dtypes

import concourse.bass as bass
import concourse.bacc as bacc
import concourse.tile as tile
from concourse import mybir
from concourse.bass_utils import run_bass_kernel_spmd

F32 = mybir.dt.float32
BF16 = mybir.dt.bfloat16
AF = mybir.ActivationFunctionType
OP = mybir.AluOpType

B, T, ENC, DEC = 256, 32, 128, 128
NCORES = 8
BS = B // NCORES   # 32 batch rows per core
BC = BS // 2       # 16 batch rows per chain
NK = T * BC // 128  # 4 tau-chunks per chain


def _ap_with(ap_obj, dims):
    """Build an AP with explicit free dims (list of [step, count]) keeping partition dim."""
    return bass.AP(tensor=ap_obj.tensor, offset=ap_obj.offset, ap=[ap_obj.ap[0]] + dims)


def build_program(n_steps=T):
    # Bacc (not plain Bass): its compile() runs move_matmul_waits_to_ldweights +
    # generate_event_semaphores, required because HW instructions hold only ONE
    # semaphore wait each.
    nc = bacc.Bacc()

    # ---- DRAM I/O (per-core shard, host-prepared layouts) ----
    d_encT = nc.dram_tensor("encT", [ENC, T * BS], F32, kind="ExternalInput")
    d_encN = nc.dram_tensor("encN", [128, 2 * NK * ENC], BF16, kind="ExternalInput")
    d_yaug = nc.dram_tensor("yaug", [2, T * BS], BF16, kind="ExternalInput")
    d_ylast = nc.dram_tensor("ylast", [1, BS], F32, kind="ExternalInput")
    d_w1eT = nc.dram_tensor("w1eT", [ENC, 128], F32, kind="ExternalInput")
    d_b1 = nc.dram_tensor("b1", [128, 1], F32, kind="ExternalInput")
    d_w1hT = nc.dram_tensor("w1hT", [DEC, 128], BF16, kind="ExternalInput")
    d_w1cT = nc.dram_tensor("w1cT", [DEC, 128], BF16, kind="ExternalInput")
    d_w2c = nc.dram_tensor("w2c", [128, 1], BF16, kind="ExternalInput")
    d_selrep = nc.dram_tensor("selrep", [128, 128], BF16, kind="ExternalInput")
    d_selb = nc.dram_tensor("selb", [128, BC], BF16, kind="ExternalInput")
    d_whhT = nc.dram_tensor("whhT", [DEC, 512], BF16, kind="ExternalInput")
    d_wcT = nc.dram_tensor("wcT", [ENC, 512], BF16, kind="ExternalInput")
    d_waug = nc.dram_tensor("waug", [2, 512], BF16, kind="ExternalInput")
    d_fcf = nc.dram_tensor("fcf", [128, 2], BF16, kind="ExternalInput")
    d_fcfb = nc.dram_tensor("fcfb", [1, 1], F32, kind="ExternalInput")
    d_out = nc.dram_tensor("outp", [1, BS], F32, kind="ExternalOutput")

    with tile.TileContext(nc) as tc:
        with (
            tc.tile_pool(name="consts", bufs=1) as consts,
            tc.tile_pool(name="state", bufs=1) as state,
            tc.tile_pool(name="temps", bufs=3) as temps,
            tc.tile_pool(name="psum", bufs=1, space="PSUM") as psum,
        ):
            # all PSUM tags are allocated as full [128, 512] banks and sliced,
            # so a tag can serve differently-shaped uses (prologue reuses gA/gB)
            def pbank(tag):
                return psum.tile([128, 512], F32, tag=tag, name="pb_" + tag)

            # ---- load constants ----
            encN = consts.tile([128, 2 * NK * ENC], BF16)
            nc.sync.dma_start(out=encN, in_=d_encN[:, :])
            yaug = consts.tile([2, T * BS], BF16)
            nc.sync.dma_start(out=yaug, in_=d_yaug[:, :])
            ylast = consts.tile([1, BS], F32)
            nc.sync.dma_start(out=ylast, in_=d_ylast[:, :])
            b1 = consts.tile([128, 1], F32)
            nc.sync.dma_start(out=b1, in_=d_b1[:, :])
            w1hT = consts.tile([DEC, 128], BF16)
            nc.sync.dma_start(out=w1hT, in_=d_w1hT[:, :])
            w1cT = consts.tile([DEC, 128], BF16)
            nc.sync.dma_start(out=w1cT, in_=d_w1cT[:, :])
            w2c = consts.tile([128, 1], BF16)
            nc.sync.dma_start(out=w2c, in_=d_w2c[:, :])
            selrep = consts.tile([128, 128], BF16)
            nc.sync.dma_start(out=selrep, in_=d_selrep[:, :])
            selb = consts.tile([128, BC], BF16)
            nc.sync.dma_start(out=selb, in_=d_selb[:, :])
            whhT = consts.tile([DEC, 512], BF16)
            nc.sync.dma_start(out=whhT, in_=d_whhT[:, :])
            wcT = consts.tile([ENC, 512], BF16)
            nc.sync.dma_start(out=wcT, in_=d_wcT[:, :])
            waug = consts.tile([2, 512], BF16)
            nc.sync.dma_start(out=waug, in_=d_waug[:, :])
            fcf = consts.tile([128, 2], BF16)
            nc.sync.dma_start(out=fcf, in_=d_fcf[:, :])
            fcfb = consts.tile([1, 1], F32)
            nc.sync.dma_start(out=fcfb, in_=d_fcfb[:, :])

            # ---- prologue: P = W1_enc @ enc + b1 (chain-major halves);
            #      F_ck = encT_ck^T @ Wc^T ----
            P = consts.tile([128, T * BS], BF16)       # [:, 512c + tau*16 + b]
            F = consts.tile([128, 2 * NK * 512], BF16)  # [:, 2048c + 512k + g]
            with tc.tile_pool(name="prolog", bufs=1) as prolog:
                encT = prolog.tile([ENC, T * BS], F32)
                nc.sync.dma_start(out=encT, in_=d_encT[:, :])
                w1eT = prolog.tile([ENC, 128], F32)
                nc.sync.dma_start(out=w1eT, in_=d_w1eT[:, :])

                # PE sync-fence: walrus Matmult/LDWEIGHTS carries at most ONE
                # semaphore wait. Touch every DMA-loaded tile with a dummy
                # 1x1x1 matmul (both operands in the same tile -> 1 wait each)
                # so no real matmul is first-contact for two sem domains.
                pdum = pbank("o")
                dscr = prolog.tile([1, 16], F32)
                for i, cst in enumerate((encT, w1eT, encN, yaug, ylast, b1, w1hT,
                                         w1cT, w2c, selrep, selb, whhT,
                                         wcT, waug, fcf, fcfb)):
                    nc.tensor.matmul(pdum[0:1, 0:1], cst[0:1, 0:1], cst[0:1, 0:1],
                                     start=True, stop=True)
                    # same fence for the vector engine (1-wait limit is universal)
                    nc.vector.tensor_copy(dscr[0:1, i:i + 1], cst[0:1, 0:1])

                # bf16 copy of encT for the F matmuls
                encTb = prolog.tile([ENC, T * BS], BF16)
                nc.vector.tensor_copy(encTb, encT[:, :])

                for half in range(2):
                    pP = pbank("gA" if half == 0 else "gB")
                    nc.tensor.matmul(
                        pP[:, :], w1eT[:, :], encT[:, half * 512:(half + 1) * 512],
                        start=True, stop=True,
                    )
                    # P half = psum + b1 (per-partition scalar), cast to bf16
                    nc.vector.tensor_scalar(
                        out=P[:, half * 512:(half + 1) * 512],
                        in0=pP[:, :], scalar1=b1[:, :], scalar2=None, op0=OP.add,
                    )

                # F_ck[r, g] = sum_e enc[b_r, tau_k(r), e] * Wc[g, e]
                # (wcT rows for i,f,o pre-scaled 0.5 host-side)
                for kk in range(2 * NK):
                    pF = pbank("gA" if kk % 2 == 0 else "gB")
                    nc.tensor.matmul(
                        pF[:, :], encTb[:, kk * 128:(kk + 1) * 128], wcT[:, :],
                        start=True, stop=True,
                    )
                    # alternate DVE/ACT for the PSUM->SBUF copies to overlap
                    if kk % 2 == 0:
                        nc.vector.tensor_copy(F[:, kk * 512:(kk + 1) * 512], pF[:, :])
                    else:
                        nc.scalar.copy(out=F[:, kk * 512:(kk + 1) * 512], in_=pF[:, :])

            # ---- per-chain state + stage emitters ----
            chains = []
            for c in range(2):
                s = dict(c=c)
                s["H"] = state.tile([DEC, BC], BF16, name="H%d" % c)
                s["T4"] = state.tile([DEC, 5 * BC], F32, name="T4s%d" % c)
                s["Cb"] = state.tile([DEC, BC], BF16, name="Cb%d" % c)
                nc.vector.memset(s["H"], 0.0)
                nc.vector.memset(s["T4"], 0.0)
                nc.vector.memset(s["Cb"], 0.0)
                s["P"] = P[:, c * 512:(c + 1) * 512]
                s["F0"] = c * 2048          # F col base
                s["encN0"] = c * NK * ENC   # encN col base
                s["yaug0"] = c * 512        # yaug col base
                chains.append(s)

            A, Bc = chains
            QT, GT, LT = ("qA", "qB"), ("gA", "gB"), ("LA", "LB")

            def st_q(s):
                # q = W1c@Cb + W1h@H -> PSUM
                pq = pbank(QT[s["c"]])
                nc.tensor.matmul(pq[:, 0:BC], w1cT[:, :], s["Cb"][:, :],
                                 start=True, stop=False)
                nc.tensor.matmul(pq[:, 0:BC], w1hT[:, :], s["H"][:, :],
                                 start=False, stop=True)
                s["pq"] = pq

            def st_qsb(s):
                q_sb = temps.tile([128, BC], BF16, tag="qsb%d" % s["c"])
                nc.vector.tensor_copy(q_sb[:, :], s["pq"][:, 0:BC])
                s["q_sb"] = q_sb
                s["pre"] = temps.tile([128, T * BC], BF16, tag="pre%d" % s["c"], name="pre%d" % s["c"])
                s["hdn"] = temps.tile([128, T * BC], BF16, tag="hdn%d" % s["c"], name="hdn%d" % s["c"])
                s["pL"] = pbank(LT[s["c"]])

            def st_pre(s, h):
                HW_ = T * BC // 2  # 256
                q_b = _ap_with(s["q_sb"][:, :], [[0, T // 2], [1, BC]])
                nc.vector.tensor_add(
                    s["pre"][:, h * HW_:(h + 1) * HW_].rearrange(
                        "p (t b) -> p t b", b=BC),
                    s["P"][:, h * HW_:(h + 1) * HW_].rearrange(
                        "p (t b) -> p t b", b=BC),
                    q_b,
                )

            def st_tanhlog(s, h):
                HW_ = T * BC // 2
                nc.scalar.activation(s["hdn"][:, h * HW_:(h + 1) * HW_],
                                     s["pre"][:, h * HW_:(h + 1) * HW_], AF.Tanh)
                for k in range(2 * h, 2 * h + 2):
                    nc.tensor.matmul(
                        s["pL"][:, k:k + 1],
                        s["hdn"][:, k * 128:(k + 1) * 128], w2c[:, :],
                        start=True, stop=True,
                    )

            def st_exp(s):
                E2 = temps.tile([128, NK], BF16, tag="E2%d" % s["c"])
                nc.scalar.activation(E2[:, :], s["pL"][:, 0:NK], AF.Exp)
                s["E2"] = E2

            def st_smm(s):
                # Sk[p, k] = sum_r [r%16==p%16]*E2[r, k] into cols NK..2NK of
                # the L bank; the k-sum happens on DVE in st_norm
                nc.tensor.matmul(s["pL"][:, NK:2 * NK], selrep[:, :],
                                 s["E2"][:, :], start=True, stop=True)

            def st_norm(s):
                abuf_u = temps.tile([128, NK * BC], BF16, tag="abufu%d" % s["c"])
                e2_b = _ap_with(s["E2"][:, :], [[1, NK], [0, BC]])
                sel_b = _ap_with(selb[:, :], [[0, NK], [1, BC]])
                nc.vector.tensor_mul(
                    abuf_u[:, :].rearrange("p (k b) -> p k b", b=BC),
                    e2_b, sel_b,
                )
                Ssum = temps.tile([128, 1], F32, tag="Ssum%d" % s["c"])
                nc.vector.tensor_reduce(Ssum[:, :], s["pL"][:, NK:2 * NK],
                                        axis=mybir.AxisListType.X, op=OP.add)
                R = temps.tile([128, 1], F32, tag="R%d" % s["c"])
                nc.vector.reciprocal(R[:, :], Ssum[:, :])
                abuf = temps.tile([128, NK * BC], BF16, tag="abuf%d" % s["c"])
                nc.vector.tensor_scalar(
                    out=abuf[:, :], in0=abuf_u[:, :], scalar1=R[:, :],
                    scalar2=None, op0=OP.mult,
                )
                s["abuf"] = abuf

            def st_gates(s, t):
                # one CONTIGUOUS accumulation group per gate block (PE allows
                # only one open group at a time -> no foreign MMs in between)
                pg = pbank(GT[s["c"]])
                y0 = s["yaug0"] + t * BC
                for m in range(4):
                    sl = pg[:, m * BC:(m + 1) * BC]
                    nc.tensor.matmul(sl, whhT[:, m * 128:(m + 1) * 128],
                                     s["H"][:, :], start=True, stop=False)
                    nc.tensor.matmul(sl, waug[:, m * 128:(m + 1) * 128],
                                     yaug[:, y0:y0 + BC], start=False, stop=False)
                    for k in range(NK):
                        nc.tensor.matmul(
                            sl,
                            F[:, s["F0"] + k * 512 + m * 128:
                                 s["F0"] + k * 512 + (m + 1) * 128],
                            s["abuf"][:, k * BC:(k + 1) * BC],
                            start=False, stop=(k == NK - 1),
                        )
                s["pg"] = pg

            def st_tanh4(s):
                # [t_o, t_i, t_f, g~] into the state tile whose tail is Cn
                nc.scalar.activation(s["T4"][:, 0:4 * BC], s["pg"][:, 0:4 * BC],
                                     AF.Tanh)

            def st_uvw(s):
                # [u|v] = (T4[i,f] + 1) * [g~|Cn] in ONE stt (layout o,i,f,g,Cn)
                T4 = s["T4"]
                uv = temps.tile([128, 2 * BC], F32, tag="uv%d" % s["c"])
                nc.vector.scalar_tensor_tensor(
                    out=uv[:, :], in0=T4[:, BC:3 * BC], scalar=1.0,
                    in1=T4[:, 3 * BC:5 * BC], op0=OP.add, op1=OP.mult)
                w2 = temps.tile([128, BC], F32, tag="w2%d" % s["c"])
                nc.vector.tensor_add(w2[:, :], uv[:, 0:BC], uv[:, BC:2 * BC])
                s["w2"] = w2

            def st_th(s):
                th = temps.tile([128, BC], F32, tag="th%d" % s["c"])
                nc.scalar.activation(th[:, :], s["w2"][:, :], AF.Tanh, scale=0.5)
                s["th"] = th

            def st_cncb(s):
                nc.vector.tensor_scalar(out=s["T4"][:, 4 * BC:5 * BC],
                                        in0=s["w2"][:, :],
                                        scalar1=0.5, scalar2=None, op0=OP.mult)
                nc.vector.tensor_scalar(out=s["Cb"][:, :], in0=s["w2"][:, :],
                                        scalar1=0.5, scalar2=None, op0=OP.mult)

            def st_H(s):
                nc.vector.scalar_tensor_tensor(
                    out=s["H"][:, :], in0=s["T4"][:, 0:BC], scalar=1.0,
                    in1=s["th"][:, :], op0=OP.add, op1=OP.mult)

            # ---- software-pipelined main loop: B runs half a step behind A ----
            for t in range(n_steps):
                st_q(A)
                if t:
                    st_gates(Bc, t - 1)
                st_qsb(A)
                if t:
                    st_tanh4(Bc)
                st_pre(A, 0)
                if t:
                    st_uvw(Bc)
                st_tanhlog(A, 0)
                if t:
                    st_th(Bc)
                st_pre(A, 1)
                if t:
                    st_cncb(Bc)
                st_tanhlog(A, 1)
                if t:
                    st_H(Bc)
                st_exp(A)
                st_q(Bc)
                st_smm(A)
                st_norm(A)
                st_qsb(Bc)
                st_gates(A, t)
                st_pre(Bc, 0)
                st_tanh4(A)
                st_tanhlog(Bc, 0)
                st_uvw(A)
                st_pre(Bc, 1)
                st_th(A)
                st_tanhlog(Bc, 1)
                st_cncb(A)
                st_H(A)
                st_exp(Bc)
                st_smm(Bc)
                st_norm(Bc)
            # drain chain B's tail for the last step
            st_gates(Bc, n_steps - 1)
            st_tanh4(Bc)
            st_uvw(Bc)
            st_th(Bc)
            st_cncb(Bc)
            st_H(Bc)

            # ---- final output: ctx for the last step from each chain's abuf ----
            out_sb = temps.tile([1, BS], F32, tag="osb")
            pctx = pbank("ctx")
            po = pbank("o")
            for s in chains:
                c = s["c"]
                csl = pctx[:, c * BC:(c + 1) * BC]
                for k in range(NK):
                    nc.tensor.matmul(
                        csl, encN[:, s["encN0"] + k * 128:s["encN0"] + (k + 1) * 128],
                        s["abuf"][:, k * BC:(k + 1) * BC],
                        start=(k == 0), stop=(k == NK - 1),
                    )
                ctx_sb = temps.tile([128, BC], BF16, tag="ctxsb%d" % c)
                nc.scalar.copy(out=ctx_sb[:, :], in_=csl)
                osl = po[0:1, c * BC:(c + 1) * BC]
                nc.tensor.matmul(osl, fcf[:, 0:1], s["H"][:, :],
                                 start=True, stop=False)
                nc.tensor.matmul(osl, fcf[:, 1:2], ctx_sb[:, :],
                                 start=False, stop=True)
                nc.vector.scalar_tensor_tensor(
                    out=out_sb[0:1, c * BC:(c + 1) * BC], in0=osl,
                    scalar=fcfb[:, :], in1=ylast[0:1, c * BC:(c + 1) * BC],
                    op0=OP.add, op1=OP.add)
            nc.sync.dma_start(out=d_out[:, :], in_=out_sb[:, :])

    nc.compile()
    return nc


def _prep_inputs(input_encoded, y_history, attn_W1, attn_b1, attn_W2, attn_b2,
                 W_ih, W_hh, b_ih, b_hh, fc_W, fc_b, fcf_W, fcf_b):
    """Host-side weight fusion + per-core shard layout prep (numpy only)."""
    f32 = np.float32
    bf16 = ml_dtypes.bfloat16
    input_encoded = np.asarray(input_encoded, f32)
    y_history = np.asarray(y_history, f32)

    # attention weights
    W1 = np.asarray(attn_W1, f32)            # [128, 384] cols: h, c, enc
    w1hT = np.ascontiguousarray((0.5 * W1[:, 0:128]).T)     # H = 2h
    w1cT = np.ascontiguousarray(W1[:, 128:256].T)
    w1eT = np.ascontiguousarray(W1[:, 256:384].T)
    b1 = np.asarray(attn_b1, f32).reshape(128, 1)
    w2c = np.asarray(attn_W2, f32).reshape(1, 128).T.copy()  # [128,1]

    # fused gate weights; reorder (i,f,g,o) -> (g,i,f,o); the i,f,o rows get
    # an extra 0.5 so one tanh produces [tanh(g), tanh(i/2), tanh(f/2), tanh(o/2)]
    W_ih = np.asarray(W_ih, f32)
    W_hh = np.asarray(W_hh, f32)
    fc_W = np.asarray(fc_W, f32)
    wc_full = np.outer(W_ih[:, 0], fc_W[0, :128])            # [512, 128]
    w_y = W_ih[:, 0] * fc_W[0, 128]
    bias_g = np.asarray(b_ih, f32) + np.asarray(b_hh, f32) + W_ih[:, 0] * f32(fc_b[0])
    perm = np.r_[384:512, 0:128, 128:256, 256:384]           # (o, i, f, g)
    gsc = np.full((512,), 0.5, f32)
    gsc[384:] = 1.0                                          # g block post-perm
    whhT = np.ascontiguousarray((0.5 * W_hh[perm] * gsc[:, None]).T)  # [128, 512]
    wcT = np.ascontiguousarray((wc_full[perm] * gsc[:, None]).T)      # [128, 512]
    waug = np.stack([w_y[perm] * gsc, bias_g[perm] * gsc], 0)         # [2, 512]

    fcf_W = np.asarray(fcf_W, f32)
    fcf = np.stack([0.5 * fcf_W[0, 0:128], fcf_W[0, 128:256]], 1)  # [128, 2]
    fcfb = np.array([[np.asarray(fcf_b, f32).reshape(-1)[0]]], f32)

    # selection matrices (per-chain, Bs=16): selb[r, b] = (r % 16 == b);
    # selrep[r, p] = (r%16 == p%16)
    r = np.arange(128)
    selb = (np.equal.outer(r % BC, np.arange(BC))).astype(f32)          # [128, 16]
    selrep = (np.equal.outer(r % BC, np.arange(128) % BC)).astype(f32)  # [128, 128]

    shared = dict(
        w1eT=w1eT, b1=b1,
        w1hT=w1hT.astype(bf16), w1cT=w1cT.astype(bf16), w2c=w2c.astype(bf16),
        selrep=selrep.astype(bf16), selb=selb.astype(bf16),
        whhT=whhT.astype(bf16), wcT=wcT.astype(bf16), waug=waug.astype(bf16),
        fcf=fcf.astype(bf16), fcfb=fcfb,
    )

    in_maps = []
    for c in range(NCORES):
        enc_c = input_encoded[c * BS:(c + 1) * BS]           # [32b, 32tau, 128e]
        y_c = y_history[c * BS:(c + 1) * BS, :, 0]           # [32b, 32tau]
        # chain-major column order: col = ch*512 + tau*16 + b16
        ec2 = enc_c.reshape(2, BC, T, ENC)                   # [ch, b16, tau, e]
        encT = np.ascontiguousarray(
            ec2.transpose(3, 0, 2, 1).reshape(ENC, T * BS))  # [e, ch*tau*b]
        # encN[r, ch*512 + k*128 + e] = enc[b16=r%16 (+16ch), tau=8k+r//16, e]
        tmp = ec2.reshape(2, BC, NK, 8, ENC)                 # [ch, b, k, tau_lo, e]
        encN = np.ascontiguousarray(
            tmp.transpose(0, 2, 3, 1, 4)                     # [ch, k, tau_lo, b, e]
            .reshape(2, NK, 128, ENC)
            .transpose(0, 2, 1, 3)                           # [ch, r, k, e]
            .reshape(2, 128, NK * ENC)
            .transpose(1, 0, 2)                              # [r, ch, k*e]
            .reshape(128, 2 * NK * ENC))
        y2 = y_c.reshape(2, BC, T)                           # [ch, b16, tau]
        yrow = np.ascontiguousarray(
            y2.transpose(0, 2, 1).reshape(1, T * BS))        # [1, ch*tau*b]
        yaug = np.concatenate([yrow, np.ones_like(yrow)], 0)  # [2, 1024]
        m = dict(shared)
        m.update(
            encT=encT, encN=encN.astype(bf16), yaug=yaug.astype(bf16),
            ylast=np.ascontiguousarray(y_c[:, T - 1].reshape(1, BS)),
        )
        in_maps.append(m)
    return in_maps


_CACHED = {}


def kernel(**inputs) -> np.ndarray:
    in_maps = _prep_inputs(**inputs)
    if "nc" not in _CACHED:
        _CACHED["nc"] = build_program()
    res = run_bass_kernel_spmd(_CACHED["nc"], in_maps, core_ids=list(range(NCORES)))
    out = np.concatenate([r["outp"].reshape(BS, 1) for r in res.results], 0)
    return out.astype(np.float32)


if __name__ == "__main__":
    import reference
    inputs = {k: np.asarray(v) for k, v in reference.setup_inputs().items()}
    expected = np.asarray(reference.reference(**inputs))
    actual = kernel(**inputs)
    err = np.abs(actual - expected).max() / (np.abs(expected).max() + 1e-12)
    print("Relative error:", err)


# revision 19
# speedup vs baseline: 1.1945x; 1.0318x over previous
"""Trainium2 Bass kernel for the attention-LSTM decoder (B=256, T-1=32, ENC=DEC=128, OUT=1).

Sharding: data-parallel, batch 256 -> 32 per core across 8 cores. On each core
the 32-batch is split into TWO independent 16-batch chains, software-pipelined
with a half-step phase offset: chain A's ACT-heavy tail (tanh4/th) overlaps
chain B's DVE/PE-heavy head (q/pre/tanh) and vice versa. The T-1=32 recurrence
is fully unrolled.

Per-chain layouts (Bs=16, r = tau_lo*16 + b with tau = 8k + tau_lo, k=0..3):
  - states: H = 2*h [128(dec), 16(b)] bf16, C [128, 16] f32 (+bf16 copy)
  - P_c = W1_enc @ enc + b1 precomputed: [128(h), 512(j=tau*16+b)] bf16
  - F_ck = encT_ck^T @ Wc^T precomputed: [128(r), 512(g)] bf16 so the gate
    contribution Wc@ctx comes straight from the attention weights abuf
  - per step: q = W1_h@h + W1_c@c (PE) -> pre = P + bcast(q) (DVE) -> tanh (ACT)
    -> logits via 4 MMs (lhsT = hdn chunk, rhs = W2) -> exp (ACT, fused row-sum)
    -> S via SEL16-matmul partition sum -> 1/S -> masked attn matrix abuf (DVE)
    -> gates = Whh@H + Waug@[y;1] + sum_k F_k@abuf_k (one contiguous PSUM
       accumulation group per gate block; PE allows only ONE open group)
    -> ONE tanh over [o|i|f|g] (o,i,f weight rows pre-scaled 0.5) into a state
       tile whose tail holds Cn, so ONE stt yields [u|v]=(T4[i,f]+1)*[g~,c];
       2c'=u+v, th=tanh(c'), H=(t_o+1)*th
"""

import numpy as np
import ml_dtypes

import concourse.bass as bass
import concourse.bacc as bacc
import concourse.tile as tile
from concourse import mybir
from concourse.bass_utils import run_bass_kernel_spmd

F32 = mybir.dt.float32
BF16 = mybir.dt.bfloat16
AF = mybir.ActivationFunctionType
OP = mybir.AluOpType

B, T, ENC, DEC = 256, 32, 128, 128
NCORES = 8
BS = B // NCORES   # 32 batch rows per core
BC = BS // 2       # 16 batch rows per chain
NK = T * BC // 128  # 4 tau-chunks per chain


def _ap_with(ap_obj, dims):
    """Build an AP with explicit free dims (list of [step, count]) keeping partition dim."""
    return bass.AP(tensor=ap_obj.tensor, offset=ap_obj.offset, ap=[ap_obj.ap[0]] + dims)


def build_program(n_steps=T):
    # Bacc (not plain Bass): its compile() runs move_matmul_waits_to_ldweights +
    # generate_event_semaphores, required because HW instructions hold only ONE
    # semaphore wait each.
    nc = bacc.Bacc()

    # ---- DRAM I/O: constants packed into 4 DMAs (HWDGE setup is 625ns each,
    # serialized -> batch aggressively) ----
    d_bb = nc.dram_tensor("bigbf", [128, 2451], BF16, kind="ExternalInput")
    d_bf = nc.dram_tensor("bigf32", [128, 1153], F32, kind="ExternalInput")
    d_yaug = nc.dram_tensor("yaug", [2, T * BS + 512], BF16, kind="ExternalInput")
    d_yl = nc.dram_tensor("yl", [1, BS + 1], F32, kind="ExternalInput")
    d_out = nc.dram_tensor("outp", [1, BS], F32, kind="ExternalOutput")

    with tile.TileContext(nc) as tc:
        with (
            tc.tile_pool(name="consts", bufs=1) as consts,
            tc.tile_pool(name="state", bufs=1) as state,
            tc.tile_pool(name="temps", bufs=3) as temps,
            tc.tile_pool(name="psum", bufs=1, space="PSUM") as psum,
        ):
            # all PSUM tags are allocated as full [128, 512] banks and sliced,
            # so a tag can serve differently-shaped uses (prologue reuses gA/gB)
            def pbank(tag):
                return psum.tile([128, 512], F32, tag=tag, name="pb_" + tag)

            # ---- load constants (4 batched DMAs) ----
            BB = consts.tile([128, 2451], BF16)
            nc.sync.dma_start(out=BB, in_=d_bb[:, :])
            YA = consts.tile([2, T * BS + 512], BF16)
            nc.sync.dma_start(out=YA, in_=d_yaug[:, :])
            YL = consts.tile([1, BS + 1], F32)
            nc.sync.dma_start(out=YL, in_=d_yl[:, :])
            # BB column map: encN 0..1024 | w1hT 1024 | w1cT 1152 | w2c 1280 |
            # selrep 1281 | selb 1409 | whhT 1425 | wcT 1937 | fcf 2449..2451
            # YA: yaug 0..1024 | waug 1024..1536 ; YL: ylast 0..32 | fcfb 32

            # ---- prologue: P = W1_enc @ enc + b1 (chain-major halves);
            #      F_ck = encT_ck^T @ Wc^T ----
            P = consts.tile([128, T * BS], BF16)       # [:, 512c + tau*16 + b]
            F = consts.tile([128, 2 * NK * 512], BF16)  # [:, 2048c + 512k + g]
            with tc.tile_pool(name="prolog", bufs=1) as prolog:
                BF = prolog.tile([128, 1153], F32)
                nc.sync.dma_start(out=BF, in_=d_bf[:, :])


                # PE sync-fence: walrus Matmult/LDWEIGHTS carries at most ONE
                # semaphore wait. Touch every DMA-loaded tile with a dummy
                # 1x1x1 matmul (both operands in the same tile -> 1 wait each)
                # so no real matmul is first-contact for two sem domains.
                pdum = pbank("o")
                dscr = prolog.tile([1, 16], F32)
                for i, cst in enumerate((BB, BF, YA, YL)):
                    nc.tensor.matmul(pdum[0:1, 0:1], cst[0:1, 0:1], cst[0:1, 0:1],
                                     start=True, stop=True)
                    # same fence for the vector engine (1-wait limit is universal)
                    nc.vector.tensor_copy(dscr[0:1, i:i + 1], cst[0:1, 0:1])

                # bf16 copy of encT for the F matmuls
                encTb = prolog.tile([ENC, T * BS], BF16)
                nc.vector.tensor_copy(encTb, BF[:, 129:1153])

                for half in range(2):
                    pP = pbank("gA" if half == 0 else "gB")
                    nc.tensor.matmul(
                        pP[:, :], BF[:, 1:129],
                        BF[:, 129 + half * 512:129 + (half + 1) * 512],
                        start=True, stop=True,
                    )
                    # P half = psum + b1 (per-partition scalar), cast to bf16
                    nc.vector.tensor_scalar(
                        out=P[:, half * 512:(half + 1) * 512],
                        in0=pP[:, :], scalar1=BF[:, 0:1], scalar2=None, op0=OP.add,
                    )

                # F_ck[r, g] = sum_e enc[b_r, tau_k(r), e] * Wc[g, e]
                # (wcT rows for i,f,o pre-scaled 0.5 host-side)
                for kk in range(2 * NK):
                    pF = pbank("gA" if kk % 2 == 0 else "gB")
                    nc.tensor.matmul(
                        pF[:, :], encTb[:, kk * 128:(kk + 1) * 128],
                        BB[:, 1937:2449],
                        start=True, stop=True,
                    )
                    # alternate DVE/ACT for the PSUM->SBUF copies to overlap
                    if kk % 2 == 0:
                        nc.vector.tensor_copy(F[:, kk * 512:(kk + 1) * 512], pF[:, :])
                    else:
                        nc.scalar.copy(out=F[:, kk * 512:(kk + 1) * 512], in_=pF[:, :])

            # ---- per-chain state + stage emitters ----
            chains = []
            for c in range(2):
                s = dict(c=c)
                s["H"] = state.tile([DEC, BC], BF16, name="H%d" % c)
                s["T4"] = state.tile([DEC, 5 * BC], F32, name="T4s%d" % c)
                s["Cb"] = state.tile([DEC, BC], BF16, name="Cb%d" % c)
                nc.vector.memset(s["H"], 0.0)
                nc.vector.memset(s["T4"], 0.0)
                nc.vector.memset(s["Cb"], 0.0)
                s["P"] = P[:, c * 512:(c + 1) * 512]
                s["F0"] = c * 2048          # F col base
                s["encN0"] = c * NK * ENC   # encN col base
                s["yaug0"] = c * 512        # yaug col base
                chains.append(s)

            A, Bc = chains
            QT, GT, LT = ("qA", "qB"), ("gA", "gB"), ("LA", "LB")

            def st_q(s):
                # q = W1c@Cb + W1h@H -> PSUM
                pq = pbank(QT[s["c"]])
                nc.tensor.matmul(pq[:, 0:BC], BB[:, 1152:1280], s["Cb"][:, :],
                                 start=True, stop=False)
                nc.tensor.matmul(pq[:, 0:BC], BB[:, 1024:1152], s["H"][:, :],
                                 start=False, stop=True)
                s["pq"] = pq

            def st_qsb(s):
                q_sb = temps.tile([128, BC], BF16, tag="qsb%d" % s["c"])
                nc.vector.tensor_copy(q_sb[:, :], s["pq"][:, 0:BC])
                s["q_sb"] = q_sb
                s["pre"] = temps.tile([128, T * BC], BF16, tag="pre%d" % s["c"], name="pre%d" % s["c"])
                s["hdn"] = temps.tile([128, T * BC], BF16, tag="hdn%d" % s["c"], name="hdn%d" % s["c"])
                s["pL"] = pbank(LT[s["c"]])

            def st_pre(s, h):
                HW_ = T * BC // 2  # 256
                q_b = _ap_with(s["q_sb"][:, :], [[0, T // 2], [1, BC]])
                nc.vector.tensor_add(
                    s["pre"][:, h * HW_:(h + 1) * HW_].rearrange(
                        "p (t b) -> p t b", b=BC),
                    s["P"][:, h * HW_:(h + 1) * HW_].rearrange(
                        "p (t b) -> p t b", b=BC),
                    q_b,
                )

            def st_tanhlog(s, h):
                HW_ = T * BC // 2
                nc.scalar.activation(s["hdn"][:, h * HW_:(h + 1) * HW_],
                                     s["pre"][:, h * HW_:(h + 1) * HW_], AF.Tanh)
                for k in range(2 * h, 2 * h + 2):
                    nc.tensor.matmul(
                        s["pL"][:, k:k + 1],
                        s["hdn"][:, k * 128:(k + 1) * 128],
                        BB[:, 1280:1281],
                        start=True, stop=True,
                    )

            def st_exp(s):
                E2 = temps.tile([128, NK], BF16, tag="E2%d" % s["c"])
                nc.scalar.activation(E2[:, :], s["pL"][:, 0:NK], AF.Exp)
                s["E2"] = E2

            def st_smm(s):
                # Sk[p, k] = sum_r [r%16==p%16]*E2[r, k] into cols NK..2NK of
                # the L bank; the k-sum happens on DVE in st_norm
                nc.tensor.matmul(s["pL"][:, NK:2 * NK], BB[:, 1281:1409],
                                 s["E2"][:, :], start=True, stop=True)

            def st_norm(s):
                abuf_u = temps.tile([128, NK * BC], BF16, tag="abufu%d" % s["c"])
                e2_b = _ap_with(s["E2"][:, :], [[1, NK], [0, BC]])
                sel_b = _ap_with(BB[:, 1409:1425], [[0, NK], [1, BC]])
                nc.vector.tensor_mul(
                    abuf_u[:, :].rearrange("p (k b) -> p k b", b=BC),
                    e2_b, sel_b,
                )
                Ssum = temps.tile([128, 1], F32, tag="Ssum%d" % s["c"])
                nc.vector.tensor_reduce(Ssum[:, :], s["pL"][:, NK:2 * NK],
                                        axis=mybir.AxisListType.X, op=OP.add)
                R = temps.tile([128, 1], F32, tag="R%d" % s["c"])
                nc.vector.reciprocal(R[:, :], Ssum[:, :])
                abuf = temps.tile([128, NK * BC], BF16, tag="abuf%d" % s["c"])
                nc.vector.tensor_scalar(
                    out=abuf[:, :], in0=abuf_u[:, :], scalar1=R[:, :],
                    scalar2=None, op0=OP.mult,
                )
                s["abuf"] = abuf

            def st_gates(s, t):
                # one CONTIGUOUS accumulation group per gate block (PE allows
                # only one open group at a time -> no foreign MMs in between)
                pg = pbank(GT[s["c"]])
                y0 = s["yaug0"] + t * BC
                for m in range(4):
                    sl = pg[:, m * BC:(m + 1) * BC]
                    nc.tensor.matmul(sl, BB[:, 1425 + m * 128:1425 + (m + 1) * 128],
                                     s["H"][:, :], start=True, stop=False)
                    nc.tensor.matmul(sl, YA[:, 1024 + m * 128:1024 + (m + 1) * 128],
                                     YA[:, y0:y0 + BC], start=False, stop=False)
                    for k in range(NK):
                        nc.tensor.matmul(
                            sl,
                            F[:, s["F0"] + k * 512 + m * 128:
                                 s["F0"] + k * 512 + (m + 1) * 128],
                            s["abuf"][:, k * BC:(k + 1) * BC],
                            start=False, stop=(k == NK - 1),
                        )
                s["pg"] = pg

            def st_tanh4(s):
                # [t_o, t_i, t_f, g~] into the state tile whose tail is Cn
                nc.scalar.activation(s["T4"][:, 0:4 * BC], s["pg"][:, 0:4 * BC],
                                     AF.Tanh)

            def st_uvw(s):
                # [u|v] = (T4[i,f] + 1) * [g~|Cn] in ONE stt (layout o,i,f,g,Cn)
                T4 = s["T4"]
                uv = temps.tile([128, 2 * BC], F32, tag="uv%d" % s["c"])
                nc.vector.scalar_tensor_tensor(
                    out=uv[:, :], in0=T4[:, BC:3 * BC], scalar=1.0,
                    in1=T4[:, 3 * BC:5 * BC], op0=OP.add, op1=OP.mult)
                w2 = temps.tile([128, BC], F32, tag="w2%d" % s["c"])
                nc.vector.tensor_add(w2[:, :], uv[:, 0:BC], uv[:, BC:2 * BC])
                s["w2"] = w2

            def st_th(s):
                th = temps.tile([128, BC], F32, tag="th%d" % s["c"])
                nc.scalar.activation(th[:, :], s["w2"][:, :], AF.Tanh, scale=0.5)
                s["th"] = th

            def st_cncb(s):
                nc.vector.tensor_scalar(out=s["T4"][:, 4 * BC:5 * BC],
                                        in0=s["w2"][:, :],
                                        scalar1=0.5, scalar2=None, op0=OP.mult)
                nc.vector.tensor_scalar(out=s["Cb"][:, :], in0=s["w2"][:, :],
                                        scalar1=0.5, scalar2=None, op0=OP.mult)

            def st_H(s):
                nc.vector.scalar_tensor_tensor(
                    out=s["H"][:, :], in0=s["T4"][:, 0:BC], scalar=1.0,
                    in1=s["th"][:, :], op0=OP.add, op1=OP.mult)

            # ---- software-pipelined main loop: B runs half a step behind A ----
            for t in range(n_steps):
                st_q(A)
                if t:
                    st_gates(Bc, t - 1)
                st_qsb(A)
                if t:
                    st_tanh4(Bc)
                st_pre(A, 0)
                if t:
                    st_uvw(Bc)
                st_tanhlog(A, 0)
                if t:
                    st_th(Bc)
                st_pre(A, 1)
                if t:
                    st_cncb(Bc)
                st_tanhlog(A, 1)
                if t:
                    st_H(Bc)
                st_exp(A)
                st_q(Bc)
                st_smm(A)
                st_norm(A)
                st_qsb(Bc)
                st_gates(A, t)
                st_pre(Bc, 0)
                st_tanh4(A)
                st_tanhlog(Bc, 0)
                st_uvw(A)
                st_pre(Bc, 1)
                st_th(A)
                st_tanhlog(Bc, 1)
                st_cncb(A)
                st_H(A)
                st_exp(Bc)
                st_smm(Bc)
                st_norm(Bc)
            # drain chain B's tail for the last step
            st_gates(Bc, n_steps - 1)
            st_tanh4(Bc)
            st_uvw(Bc)
            st_th(Bc)
            st_cncb(Bc)
            st_H(Bc)

            # ---- final output: ctx for the last step from each chain's abuf ----
            out_sb = temps.tile([1, BS], F32, tag="osb")
            pctx = pbank("ctx")
            po = pbank("o")
            for s in chains:
                c = s["c"]
                csl = pctx[:, c * BC:(c + 1) * BC]
                for k in range(NK):
                    nc.tensor.matmul(
                        csl, BB[:, s["encN0"] + k * 128:s["encN0"] + (k + 1) * 128],
                        s["abuf"][:, k * BC:(k + 1) * BC],
                        start=(k == 0), stop=(k == NK - 1),
                    )
                ctx_sb = temps.tile([128, BC], BF16, tag="ctxsb%d" % c)
                nc.scalar.copy(out=ctx_sb[:, :], in_=csl)
                osl = po[0:1, c * BC:(c + 1) * BC]
                nc.tensor.matmul(osl, BB[:, 2449:2450], s["H"][:, :],
                                 start=True, stop=False)
                nc.tensor.matmul(osl, BB[:, 2450:2451], ctx_sb[:, :],
                                 start=False, stop=True)
                nc.vector.scalar_tensor_tensor(
                    out=out_sb[0:1, c * BC:(c + 1) * BC], in0=osl,
                    scalar=YL[0:1, BS:BS + 1], in1=YL[0:1, c * BC:(c + 1) * BC],
                    op0=OP.add, op1=OP.add)
            nc.sync.dma_start(out=d_out[:, :], in_=out_sb[:, :])

    nc.compile()
    return nc


def _prep_inputs(input_encoded, y_history, attn_W1, attn_b1, attn_W2, attn_b2,
                 W_ih, W_hh, b_ih, b_hh, fc_W, fc_b, fcf_W, fcf_b):
    """Host-side weight fusion + per-core shard layout prep (numpy only)."""
    f32 = np.float32
    bf16 = ml_dtypes.bfloat16
    input_encoded = np.asarray(input_encoded, f32)
    y_history = np.asarray(y_history, f32)

    # attention weights
    W1 = np.asarray(attn_W1, f32)            # [128, 384] cols: h, c, enc
    w1hT = np.ascontiguousarray((0.5 * W1[:, 0:128]).T)     # H = 2h
    w1cT = np.ascontiguousarray(W1[:, 128:256].T)
    w1eT = np.ascontiguousarray(W1[:, 256:384].T)
    b1 = np.asarray(attn_b1, f32).reshape(128, 1)
    w2c = np.asarray(attn_W2, f32).reshape(1, 128).T.copy()  # [128,1]

    # fused gate weights; reorder (i,f,g,o) -> (g,i,f,o); the i,f,o rows get
    # an extra 0.5 so one tanh produces [tanh(g), tanh(i/2), tanh(f/2), tanh(o/2)]
    W_ih = np.asarray(W_ih, f32)
    W_hh = np.asarray(W_hh, f32)
    fc_W = np.asarray(fc_W, f32)
    wc_full = np.outer(W_ih[:, 0], fc_W[0, :128])            # [512, 128]
    w_y = W_ih[:, 0] * fc_W[0, 128]
    bias_g = np.asarray(b_ih, f32) + np.asarray(b_hh, f32) + W_ih[:, 0] * f32(fc_b[0])
    perm = np.r_[384:512, 0:128, 128:256, 256:384]           # (o, i, f, g)
    gsc = np.full((512,), 0.5, f32)
    gsc[384:] = 1.0                                          # g block post-perm
    whhT = np.ascontiguousarray((0.5 * W_hh[perm] * gsc[:, None]).T)  # [128, 512]
    wcT = np.ascontiguousarray((wc_full[perm] * gsc[:, None]).T)      # [128, 512]
    waug = np.stack([w_y[perm] * gsc, bias_g[perm] * gsc], 0)         # [2, 512]

    fcf_W = np.asarray(fcf_W, f32)
    fcf = np.stack([0.5 * fcf_W[0, 0:128], fcf_W[0, 128:256]], 1)  # [128, 2]
    fcfb = np.array([[np.asarray(fcf_b, f32).reshape(-1)[0]]], f32)

    # selection matrices (per-chain, Bs=16): selb[r, b] = (r % 16 == b);
    # selrep[r, p] = (r%16 == p%16)
    r = np.arange(128)
    selb = (np.equal.outer(r % BC, np.arange(BC))).astype(f32)          # [128, 16]
    selrep = (np.equal.outer(r % BC, np.arange(128) % BC)).astype(f32)  # [128, 128]

    # bf16 packed block: encN(1024) w1hT(128) w1cT(128) w2c(1) selrep(128)
    # selb(16) whhT(512) wcT(512) fcf(2) -> per-core (encN differs per core)
    bb_shared = np.concatenate([
        w1hT, w1cT, w2c, selrep, selb, whhT, wcT, fcf], 1).astype(bf16)
    shared = dict(bb_shared=bb_shared)

    in_maps = []
    for c in range(NCORES):
        enc_c = input_encoded[c * BS:(c + 1) * BS]           # [32b, 32tau, 128e]
        y_c = y_history[c * BS:(c + 1) * BS, :, 0]           # [32b, 32tau]
        # chain-major column order: col = ch*512 + tau*16 + b16
        ec2 = enc_c.reshape(2, BC, T, ENC)                   # [ch, b16, tau, e]
        encT = np.ascontiguousarray(
            ec2.transpose(3, 0, 2, 1).reshape(ENC, T * BS))  # [e, ch*tau*b]
        # encN[r, ch*512 + k*128 + e] = enc[b16=r%16 (+16ch), tau=8k+r//16, e]
        tmp = ec2.reshape(2, BC, NK, 8, ENC)                 # [ch, b, k, tau_lo, e]
        encN = np.ascontiguousarray(
            tmp.transpose(0, 2, 3, 1, 4)                     # [ch, k, tau_lo, b, e]
            .reshape(2, NK, 128, ENC)
            .transpose(0, 2, 1, 3)                           # [ch, r, k, e]
            .reshape(2, 128, NK * ENC)
            .transpose(1, 0, 2)                              # [r, ch, k*e]
            .reshape(128, 2 * NK * ENC))
        y2 = y_c.reshape(2, BC, T)                           # [ch, b16, tau]
        yrow = np.ascontiguousarray(
            y2.transpose(0, 2, 1).reshape(1, T * BS))        # [1, ch*tau*b]
        yaug = np.concatenate(
            [np.concatenate([yrow, np.ones_like(yrow)], 0),
             waug.astype(f32)], 1)                           # [2, 1024+512]
        bigbf = np.concatenate([encN.astype(bf16), bb_shared], 1)
        bigf32 = np.concatenate(
            [b1, w1eT, encT], 1).astype(f32)
        yl = np.concatenate(
            [y_c[:, T - 1].reshape(1, BS), fcfb.reshape(1, 1)], 1).astype(f32)
        m = dict(
            bigbf=np.ascontiguousarray(bigbf),
            bigf32=np.ascontiguousarray(bigf32),
            yaug=yaug.astype(bf16),
            yl=np.ascontiguousarray(yl),
        )
        in_maps.append(m)
    return in_maps


_CACHED = {}


def kernel(**inputs) -> np.ndarray:
    in_maps = _prep_inputs(**inputs)
    if "nc" not in _CACHED:
        _CACHED["nc"] = build_program()
    res = run_bass_kernel_spmd(_CACHED["nc"], in_maps, core_ids=list(range(NCORES)))
    out = np.concatenate([r["outp"].reshape(BS, 1) for r in res.results], 0)
    return out.astype(np.float32)


if __name__ == "__main__":
    import reference
    inputs = {k: np.asarray(v) for k, v in reference.setup_inputs().items()}
    expected = np.asarray(reference.reference(**inputs))
    actual = kernel(**inputs)
    err = np.abs(actual - expected).max() / (np.abs(expected).max() + 1e-12)
    print("Relative error:", err)
